# revision 1
# baseline (speedup 1.0000x reference)
"""BitStackLinear Trainium2 kernel.

Computes out = x @ w.T where w = sum_i sign_i * (u_i @ vt_i), signs unpacked
from 4 packed bit-planes (one byte = 8 signs, little-endian).

Strategy: tensor-parallel over out_features across 8 NeuronCores
(1376 rows each). Per core, on device:

  Phase R (reconstruct w.T shard [4096, 1376] into DRAM, per 128-row k-slab):
    - PE: r_i = vt_i.T @ u_i.T (rank-16 fp32r matmuls) -> PSUM
    - ScalarE: r2_i = psum->SBUF copy with per-partition scale 2^(1-j), j=p%8
    - DMA: packed sign bytes broadcast 8x across partitions
    - GpSimd: a_i = bytes & (1<<j)  in {0, 2^j}
    - DVE: t_i = (a_i - 2^(j-1)) * r2_i = sign_i * r_i ; acc += t_i
  Phase G (GEMM out.T = w.T^T-contraction, fp32r):
    - x.T chunk [4096, 1024] resident in SBUF (moving operand)
    - w.T tiles streamed from DRAM once per m-block (stationary operand)
    - PSUM accumulation over k (32 x 128), ScalarE evacuation, DMA out

kernel(**inputs) takes the full unsharded inputs and returns the full output.
Host work is layout only: transposes, dtype reinterpretation, sharding.
"""

import numpy as np

import concourse.bass as bass
import concourse.bacc as bacc
import concourse.mybir as mybir
import concourse.tile as tile

W_BIT = 4
OUT_F = 11008
IN_F = 4096
RANK = 16
NCORES = 8
O_SHARD = OUT_F // NCORES          # 1376
O_TILES = (O_SHARD + 127) // 128   # 11 (last tile 96 wide)
K_TILES = IN_F // 128              # 32
MB = 1024                          # m-block (resident x.T chunk width)


def _bitstack_body(tc, aps, M):
    nc = tc.nc
    xT, qbT, uT, vt, bm, hm, pps, wt_d, outT = (
        aps["xT"], aps["qbT"], aps["uT"], aps["vt"], aps["bm"], aps["hm"],
        aps["pps"], aps["wt_d"], aps["outT"],
    )
    f32, u8, i32 = mybir.dt.float32, mybir.dt.uint8, mybir.dt.int32
    f32r = mybir.dt.float32r
    n_mb = M // MB

    import contextlib
    with contextlib.ExitStack() as ctx:
        pool = ctx.enter_context(tc.tile_pool(name="sb", bufs=1))
        psum = ctx.enter_context(tc.tile_pool(name="ps", bufs=2, space="PSUM"))

        # ---- constants resident in SBUF ----
        bm_t = pool.tile([128, O_SHARD], u8, name="bm_t")
        nc.sync.dma_start(bm_t, bm)
        hm_t = pool.tile([128, 1], f32, name="hm_t")
        nc.sync.dma_start(hm_t, hm)
        pps_t = pool.tile([128, 1], f32, name="pps_t")
        nc.sync.dma_start(pps_t, pps)
        # prefetch m-block 0's x chunk during recon (no deps on recon)
        xk0 = []
        for k in range(K_TILES):
            t = pool.tile([128, MB], f32r, name=f"xk0_{k}", tag="xk", bufs=34)
            nc.sync.dma_start(t, xT[k * 128:(k + 1) * 128, 0:MB].bitcast(f32r))
            xk0.append(t)

        # ---- Phase R: reconstruct w.T k-slabs into wt_d ----
        for ks in range(K_TILES):
            acc = pool.tile([128, O_SHARD], f32, name=f"acc{ks}", tag="acc", bufs=2)
            for i in range(W_BIT):
                # vt slice [16, 128] and u.T [16, O] for this (slab, bit)
                vtb = pool.tile([16, 128], f32r, name=f"vtb{ks}_{i}", tag="vtb", bufs=4)
                nc.sync.dma_start(vtb, vt[i, :, ks * 128:(ks + 1) * 128].bitcast(f32r))
                utb = pool.tile([16, O_SHARD], f32r, name=f"utb{ks}_{i}", tag="utb", bufs=2)
                nc.sync.dma_start(utb, uT[i].bitcast(f32r))
                # r_i = vt_i.T @ u_i.T -> psum chunks (single-bank tiles), then
                # r2 = psum -> sbuf with per-partition scale 2^(1-j)
                r2 = pool.tile([128, O_SHARD], f32, name=f"r2_{ks}_{i}", tag="r2", bufs=2)
                for ci, c0 in enumerate(range(0, O_SHARD, 512)):
                    c1 = min(c0 + 512, O_SHARD)
                    pr = psum.tile([128, 512], f32, name=f"pr{ks}_{i}_{ci}", tag="ps", bufs=6)
                    nc.tensor.matmul(
                        pr[:, :c1 - c0], vtb,
                        utb[:, c0:c1],
                        start=True, stop=True,
                    )
                    nc.scalar.activation(r2[:, c0:c1], pr[:, :c1 - c0],
                                         mybir.ActivationFunctionType.Copy,
                                         scale=pps_t)
                # packed bytes, broadcast 8x along partitions
                bts = pool.tile([128, O_SHARD], u8, name=f"bts{ks}_{i}", tag="bts", bufs=2)
                src = qbT[i, ks * 16:(ks + 1) * 16][:, None, :].to_broadcast(
                    (16, 8, O_SHARD))
                nc.sync.dma_start(bts, src)
                # a = bytes & bitmask -> {0, 2^j}; AND runs on DVE over int32
                # views (4 packed bytes/lane/cycle; bitwise ops are DVE+i32 only)
                a_t = pool.tile([128, O_SHARD], u8, name=f"a{ks}_{i}", tag="a", bufs=2)
                nc.vector.tensor_tensor(out=a_t.bitcast(i32), in0=bts.bitcast(i32),
                                        in1=bm_t.bitcast(i32),
                                        op=mybir.AluOpType.bitwise_and)
                # t = (a - 2^(j-1)) * r2 = sign * r  (DVE); accumulate on GpSimd
                if i == 0:
                    nc.vector.scalar_tensor_tensor(
                        out=acc, in0=a_t, scalar=hm_t, in1=r2,
                        op0=mybir.AluOpType.subtract, op1=mybir.AluOpType.mult)
                else:
                    t_t = pool.tile([128, O_SHARD], f32, name=f"t{ks}_{i}", tag="tt",
                                    bufs=1)
                    nc.vector.scalar_tensor_tensor(
                        out=t_t, in0=a_t, scalar=hm_t, in1=r2,
                        op0=mybir.AluOpType.subtract, op1=mybir.AluOpType.mult)
                    nc.vector.tensor_tensor(out=acc, in0=acc, in1=t_t,
                                            op=mybir.AluOpType.add)
            # store slab to wt_d[ot][:, ks, :]
            for ot in range(O_TILES):
                ow = min(128, O_SHARD - ot * 128)
                nc.sync.dma_start(wt_d[ot, :, ks, :ow],
                                  acc[:, ot * 128:ot * 128 + ow])

        # ---- Phase G: out.T[o, m] = sum_k wT[k, o] * xT[k, m] ----
        for mb in range(n_mb):
            if mb == 0:
                xk = xk0
            else:
                xk = []
                for k in range(K_TILES):
                    t = pool.tile([128, MB], f32r, name=f"xk{mb}_{k}", tag="xk",
                                  bufs=34)
                    nc.sync.dma_start(t, xT[k * 128:(k + 1) * 128,
                                            mb * MB:(mb + 1) * MB].bitcast(f32r))
                    xk.append(t)
            for ot in range(O_TILES):
                ow = min(128, O_SHARD - ot * 128)
                # stream w.T k-column for this o-tile in two halves
                wc = []
                for kh in range(2):
                    t = pool.tile([128, 16, 128], f32r, name=f"wc{mb}_{ot}_{kh}",
                                  tag="wc", bufs=2)
                    nc.sync.dma_start(t[:, :, :ow],
                                      wt_d[ot, :, kh * 16:(kh + 1) * 16, :ow]
                                      .bitcast(f32r))
                    wc.append(t)
                # two 512-m psum groups accumulated together; halves
                # interleaved per k so consecutive matmuls share the same
                # stationary tile (walrus ldw-opt dedups the reload)
                nh = MB // 512
                pss = [psum.tile([128, 512], f32, name=f"g{mb}_{ot}_{h}",
                                 tag="ps", bufs=6) for h in range(nh)]
                for k in range(K_TILES):
                    for h in range(nh):
                        nc.tensor.matmul(
                            pss[h][:ow],
                            wc[k // 16][:, k % 16, :ow],
                            xk[k][:, h * 512:(h + 1) * 512],
                            start=(k == 0), stop=(k == K_TILES - 1),
                        )
                for h in range(nh):
                    ost = pool.tile([128, 512], f32, name=f"ost{mb}_{ot}_{h}",
                                    tag="ost", bufs=2)
                    nc.scalar.copy(ost[:ow], pss[h][:ow])
                    nc.sync.dma_start(
                        outT[ot * 128:ot * 128 + ow,
                             mb * MB + h * 512: mb * MB + (h + 1) * 512],
                        ost[:ow])


def build_bass(M=8192):
    nc = bacc.Bacc("TRN2", target_bir_lowering=False, debug=False)
    f32, u8 = mybir.dt.float32, mybir.dt.uint8
    aps = {}
    aps["xT"] = nc.dram_tensor("xT", [IN_F, M], f32, kind="ExternalInput").ap()
    aps["qbT"] = nc.dram_tensor("qbT", [W_BIT, IN_F // 8, O_SHARD], u8,
                                kind="ExternalInput").ap()
    aps["uT"] = nc.dram_tensor("uT", [W_BIT, RANK, O_SHARD], f32,
                               kind="ExternalInput").ap()
    aps["vt"] = nc.dram_tensor("vt", [W_BIT, RANK, IN_F], f32,
                               kind="ExternalInput").ap()
    aps["bm"] = nc.dram_tensor("bm", [128, O_SHARD], u8, kind="ExternalInput").ap()
    aps["hm"] = nc.dram_tensor("hm", [128, 1], f32, kind="ExternalInput").ap()
    aps["pps"] = nc.dram_tensor("pps", [128, 1], f32, kind="ExternalInput").ap()
    aps["wt_d"] = nc.dram_tensor("wt_d", [O_TILES, 128, K_TILES, 128], f32,
                                 kind="Internal").ap()
    aps["outT"] = nc.dram_tensor("outT", [O_SHARD, M], f32,
                                 kind="ExternalOutput").ap()
    with tile.TileContext(nc) as tc:
        _bitstack_body(tc, aps, M)
    nc.compile()
    return nc


def prep_inputs(x, qweight, u, vt):
    """Host-side layout prep (transposes / dtype views / sharding only)."""
    M = x.shape[0] * x.shape[1]
    xT = np.ascontiguousarray(x.reshape(M, IN_F).T)
    qb = qweight.astype(np.uint8)  # values 0..255 stored in int32
    p = np.arange(128)
    bm = (np.uint8(1) << (p % 8).astype(np.uint8))[:, None] * np.ones(
        (1, O_SHARD), np.uint8)
    hm = (2.0 ** ((p % 8) - 1.0)).astype(np.float32).reshape(128, 1)
    pps = (2.0 ** (1.0 - (p % 8))).astype(np.float32).reshape(128, 1)
    vt_c = np.ascontiguousarray(vt)
    in_maps = []
    for c in range(NCORES):
        sl = slice(c * O_SHARD, (c + 1) * O_SHARD)
        qbT = np.ascontiguousarray(
            qb.reshape(W_BIT, OUT_F, IN_F // 8)[:, sl, :].transpose(0, 2, 1))
        uT = np.ascontiguousarray(u[:, sl, :].transpose(0, 2, 1))
        in_maps.append({
            "xT": xT, "qbT": qbT, "uT": uT, "vt": vt_c,
            "bm": bm, "hm": hm, "pps": pps,
        })
    return in_maps


def _enable_ldw_opt():
    """Rewrite our walrus invocation to enable redundant-LDWEIGHTS
    elimination (consecutive matmuls sharing a stationary tile skip the
    reload)."""
    from concourse import bass_utils as bu
    if getattr(bu, "_ldw_opt_patched", False):
        return
    orig = bu.run_command

    def patched(argv, **kw):
        argv = ["--enable-ldw-opt=true" if a == "--enable-ldw-opt=false" else a
                for a in argv]
        return orig(argv, **kw)

    bu.run_command = patched
    bu._ldw_opt_patched = True


def kernel(x, qweight, u, vt):
    from concourse import bass_utils
    _enable_ldw_opt()
    x = np.asarray(x)
    qweight = np.asarray(qweight)
    u = np.asarray(u)
    vt = np.asarray(vt)
    B, S, _ = x.shape
    M = B * S
    nc = build_bass(M)
    in_maps = prep_inputs(x, qweight, u, vt)
    res = bass_utils.run_bass_kernel_spmd(nc, in_maps, core_ids=list(range(NCORES)))
    out = np.empty((M, OUT_F), np.float32)
    for c in range(NCORES):
        out[:, c * O_SHARD:(c + 1) * O_SHARD] = res.results[c]["outT"].T
    return out.reshape(B, S, OUT_F)


if __name__ == "__main__":
    # smoke test at small M via CoreSim is in sim_test.py; here run full HW
    rng = np.random.default_rng(0)
    x = rng.standard_normal((4, 2048, IN_F)).astype(np.float32)
    qw = rng.integers(0, 256, size=(W_BIT, OUT_F * IN_F // 8)).astype(np.int32)
    uu = (rng.standard_normal((W_BIT, OUT_F, RANK)) * 0.05).astype(np.float32)
    vv = (rng.standard_normal((W_BIT, RANK, IN_F)) * 0.05).astype(np.float32)
    out = kernel(x=x, qweight=qw, u=uu, vt=vv)
    print(out.shape, out.dtype)



# revision 8
# speedup vs baseline: 1.3231x; 1.3231x over previous
"""BitStackLinear Trainium2 kernel (v2: bf16 GEMM with SBUF-resident w.T).

Computes out = x @ w.T where w = sum_i sign_i * (u_i @ vt_i), signs unpacked
from 4 packed bit-planes (one byte = 8 signs, little-endian).

Strategy: tensor-parallel over out_features across 8 NeuronCores
(1376 rows each). Per core, the o-dim is split into chunk A (4 o-tiles,
512 cols) and chunk B (7 o-tiles, 864 cols) so that reconstruction of B
overlaps the GEMM over A:

  [recon A] -> [GEMM-A over all m  ||  recon B] -> [GEMM-B over all m]

Reconstruction of w.T chunk (per 128-row k-slab, bf16, RESIDENT in SBUF):
  - DMA: vt k-slices (4 bits packed in one [16,512] tile); packed sign
    bytes broadcast 8x across partitions (4 bits side by side)
  - GpSimd: vtb4s = vtb4 * pat (folds the 2^(1-j) per-k scale, j=k%8);
    a4 = bytes4 & (1<<j) in {0, 2^j} (one i32 AND for all 4 bits)
  - PE: pr_i = vtb4s_i.T @ u_i.T -> PSUM f32 (rank-16 matmuls)
  - DVE: t_i = (a_i - 2^(j-1)) * pr_i = sign_i * r_i (STT, bf16 out)
  - PE: acc += I.T @ t_i (identity matmuls accumulate the 4 bit-planes in
    f32 PSUM; replaces 3 DVE adds)
  - ScalarE: wt[ks] = acc (evacuate to the resident bf16 w.T tile)
GEMM (all-bf16 PE, PSUM accumulation over all 32 k-slabs):
  - x.T streamed f32 per 512-col m-block, cast to bf16 (ScalarE/DVE/GpSimd)
  - stationary = resident wt[k] column tiles (bf16 -> FWL hides LDWEIGHTS)
  - ScalarE evacuation, DMA out

kernel(**inputs) takes the full unsharded inputs and returns the full output.
Host work is layout only: transposes, dtype reinterpretation, sharding.
"""

import numpy as np

import concourse.bass as bass
import concourse.bacc as bacc
import concourse.mybir as mybir
import concourse.tile as tile

W_BIT = 4
OUT_F = 11008
IN_F = 4096
RANK = 16
NCORES = 8
O_SHARD = OUT_F // NCORES          # 1376
O_TILES = (O_SHARD + 127) // 128   # 11 (last tile 96 wide)
K_TILES = IN_F // 128              # 32
MB = 512                           # m-block (x chunk width)
OT_A = 4                           # o-tiles in chunk A
O_A = OT_A * 128                   # 512
O_B = O_SHARD - O_A                # 864


def _bitstack_body(tc, aps, M):
    nc = tc.nc
    xT, qbT, uT, vt, bm, hm, pat, idn, outT = (
        aps["xT"], aps["qbT"], aps["uT"], aps["vt"], aps["bm"], aps["hm"],
        aps["pat"], aps["idn"], aps["outT"],
    )
    f32, u8, i32 = mybir.dt.float32, mybir.dt.uint8, mybir.dt.int32
    bf16 = mybir.dt.bfloat16
    n_mb = M // MB

    import contextlib
    with contextlib.ExitStack() as ctx:
        pool = ctx.enter_context(tc.tile_pool(name="sb", bufs=1))
        psum = ctx.enter_context(tc.tile_pool(name="ps", bufs=1, space="PSUM"))

        # ---- constants resident in SBUF ----
        bm_t = pool.tile([128, W_BIT * O_B], u8, name="bm_t")
        nc.sync.dma_start(bm_t, bm)
        hm_t = pool.tile([128, 1], f32, name="hm_t")
        nc.sync.dma_start(hm_t, hm)
        pat_t = pool.tile([16, 512], f32, name="pat_t")
        nc.sync.dma_start(pat_t, pat)
        idf_t = pool.tile([128, 128], f32, name="idf_t")
        nc.sync.dma_start(idf_t, idn)
        idn_t = pool.tile([128, 128], bf16, name="idn_t")
        nc.scalar.copy(idn_t, idf_t)

        # u.T resident in bf16: staged per 512-chunk through a small f32 tile
        utb = []
        for i in range(W_BIT):
            t = pool.tile([16, O_SHARD], bf16, name=f"utb{i}", tag="utb", bufs=4)
            for c0 in range(0, O_SHARD, 512):
                c1 = min(c0 + 512, O_SHARD)
                st = pool.tile([16, 512], f32, name=f"ust{i}_{c0}", tag="ost",
                               bufs=2)
                nc.sync.dma_start(st[:, :c1 - c0], uT[i, :, c0:c1])
                nc.scalar.copy(t[:, c0:c1], st[:, :c1 - c0])
            utb.append(t)

        # ---- x streaming helpers (DMA f32 chunk, cast to bf16) ----
        xbl = {}

        def emit_x(pas, mb, k, eng):
            xs = pool.tile([128, MB], f32, name=f"xs{pas}{mb}_{k}", tag="xs",
                           bufs=4)
            nc.sync.dma_start(xs, xT[k * 128:(k + 1) * 128,
                                     mb * MB:(mb + 1) * MB])
            xb = pool.tile([128, MB], bf16, name=f"xb{pas}{mb}_{k}", tag="xb",
                           bufs=60)
            if eng == "s":
                nc.scalar.copy(xb, xs)
            elif eng == "v":
                nc.vector.tensor_copy(out=xb, in_=xs)
            else:
                nc.gpsimd.tensor_copy(out=xb, in_=xs)
            xbl[(pas, mb, k)] = xb

        def recon_slab(ks, o0, ow, wtag, dmae):
            """Reconstruct w.T[ks*128:(ks+1)*128, o0:o0+ow] -> bf16 tile."""
            vtb4 = pool.tile([16, 512], f32, name=f"vtb4_{wtag}{ks}",
                             tag="vtb4", bufs=2)
            for i in range(W_BIT):
                dmae.dma_start(vtb4[:, i * 128:(i + 1) * 128],
                               vt[i, :, ks * 128:(ks + 1) * 128])
            vtb4s = pool.tile([16, 512], bf16, name=f"vtb4s_{wtag}{ks}",
                              tag="vtb4s", bufs=2)
            nc.gpsimd.tensor_tensor(out=vtb4s, in0=vtb4, in1=pat_t,
                                    op=mybir.AluOpType.mult)
            bts4 = pool.tile([128, W_BIT * ow], u8, name=f"bts{wtag}{ks}",
                             tag=f"bts{wtag}", bufs=1)
            for i in range(W_BIT):
                src = qbT[i, ks * 16:(ks + 1) * 16,
                          o0:o0 + ow][:, None, :].to_broadcast((16, 8, ow))
                dmae.dma_start(bts4[:, i * ow:(i + 1) * ow], src)
            a4 = pool.tile([128, W_BIT * ow], u8, name=f"a{wtag}{ks}",
                           tag=f"a{wtag}", bufs=2)
            nc.vector.tensor_tensor(out=a4.bitcast(i32), in0=bts4.bitcast(i32),
                                    in1=bm_t.bitcast(i32)[:, 0:W_BIT * ow // 4],
                                    op=mybir.AluOpType.bitwise_and)
            acc = psum.tile([128, O_B], f32, name=f"acc{wtag}{ks}", tag="acc",
                            bufs=1)
            chunks = [(c0, min(c0 + 512, ow)) for c0 in range(0, ow, 512)]
            for i in range(W_BIT):
                pr = psum.tile([128, O_B], f32, name=f"pr{wtag}{ks}_{i}",
                               tag="pr", bufs=2)
                for c0, c1 in chunks:
                    nc.tensor.matmul(pr[:, c0:c1],
                                     vtb4s[:, i * 128:(i + 1) * 128],
                                     utb[i][:, o0 + c0:o0 + c1],
                                     start=True, stop=True)
                t_t = pool.tile([128, O_B], bf16, name=f"t{wtag}{ks}_{i}",
                                tag="tt", bufs=2)
                nc.vector.scalar_tensor_tensor(
                    out=t_t[:, :ow], in0=a4[:, i * ow:(i + 1) * ow],
                    scalar=hm_t, in1=pr[:, :ow],
                    op0=mybir.AluOpType.subtract, op1=mybir.AluOpType.mult)
                for c0, c1 in chunks:
                    nc.tensor.matmul(acc[:, c0:c1], idn_t, t_t[:, c0:c1],
                                     start=(i == 0), stop=(i == W_BIT - 1))
            w = pool.tile([128, ow], bf16, name=f"wt{wtag}{ks}", tag=f"wt{wtag}",
                          bufs=K_TILES)
            nc.scalar.copy(w, acc[:, :ow])
            return w

        def gemm_mb(pas, mb, ots, wtl, o_base):
            for ot in ots:
                ow = min(128, O_SHARD - ot * 128)
                c0 = ot * 128 - o_base
                pg = psum.tile([128, MB], f32, name=f"g{pas}{mb}_{ot}",
                               tag="pg", bufs=2)
                for k in range(K_TILES):
                    nc.tensor.matmul(
                        pg[:ow],
                        wtl[k][:, c0:c0 + ow],
                        xbl[(pas, mb, k)],
                        start=(k == 0), stop=(k == K_TILES - 1),
                    )
                ost = pool.tile([128, MB], f32, name=f"ost{pas}{mb}_{ot}",
                                tag="ost", bufs=2)
                nc.scalar.copy(ost[:ow], pg[:ow])
                nc.sync.dma_start(
                    outT[ot * 128:ot * 128 + ow, mb * MB:(mb + 1) * MB],
                    ost[:ow])

        # ---- Phase R-A: reconstruct w.T chunk A; sprinkle mb0/mb1 x prep ----
        cast_jobs = [("A", mb, k) for mb in (0, 1) for k in range(K_TILES)]
        wtA = []
        for ks in range(K_TILES):
            wtA.append(recon_slab(ks, 0, O_A, "A", nc.sync))
            if cast_jobs:
                emit_x(*cast_jobs.pop(0), "s")
            if cast_jobs:
                emit_x(*cast_jobs.pop(0), "g")

        # ---- Phase G-A x R-B: GEMM chunk A overlapping recon of chunk B ----
        wtB = []
        for mb in range(n_mb):
            if mb + 2 < n_mb:
                for k in range(K_TILES):
                    emit_x("A", mb + 2, k, "s" if k % 4 != 3 else "g")
            gemm_mb("A", mb, range(OT_A), wtA, 0)
            for s in (2 * mb, 2 * mb + 1):
                if s < K_TILES:
                    wtB.append(recon_slab(s, O_A, O_B, "B", nc.gpsimd))
        while len(wtB) < K_TILES:
            wtB.append(recon_slab(len(wtB), O_A, O_B, "B", nc.gpsimd))

        # ---- Phase G-B: GEMM chunk B ----
        for k in range(K_TILES):
            emit_x("B", 0, k, "s" if k % 2 == 0 else "v")
        for k in range(K_TILES):
            emit_x("B", 1, k, "s" if k % 2 == 0 else "v")
        for mb in range(n_mb):
            if mb + 2 < n_mb:
                for k in range(K_TILES):
                    emit_x("B", mb + 2, k, "s" if k % 2 == 0 else "v")
            gemm_mb("B", mb, range(OT_A, O_TILES), wtB, O_A)


def build_bass(M=8192):
    nc = bacc.Bacc("TRN2", target_bir_lowering=False, debug=False)
    f32, u8 = mybir.dt.float32, mybir.dt.uint8
    aps = {}
    aps["xT"] = nc.dram_tensor("xT", [IN_F, M], f32, kind="ExternalInput").ap()
    aps["qbT"] = nc.dram_tensor("qbT", [W_BIT, IN_F // 8, O_SHARD], u8,
                                kind="ExternalInput").ap()
    aps["uT"] = nc.dram_tensor("uT", [W_BIT, RANK, O_SHARD], f32,
                               kind="ExternalInput").ap()
    aps["vt"] = nc.dram_tensor("vt", [W_BIT, RANK, IN_F], f32,
                               kind="ExternalInput").ap()
    aps["bm"] = nc.dram_tensor("bm", [128, W_BIT * O_B], u8,
                               kind="ExternalInput").ap()
    aps["hm"] = nc.dram_tensor("hm", [128, 1], f32, kind="ExternalInput").ap()
    aps["pat"] = nc.dram_tensor("pat", [16, 512], f32, kind="ExternalInput").ap()
    aps["idn"] = nc.dram_tensor("idn", [128, 128], f32, kind="ExternalInput").ap()
    aps["outT"] = nc.dram_tensor("outT", [O_SHARD, M], f32,
                                 kind="ExternalOutput").ap()
    with tile.TileContext(nc) as tc:
        _bitstack_body(tc, aps, M)
    nc.compile()
    return nc


def prep_inputs(x, qweight, u, vt):
    """Host-side layout prep (transposes / dtype views / sharding only)."""
    M = x.shape[0] * x.shape[1]
    xT = np.ascontiguousarray(x.reshape(M, IN_F).T)
    qb = qweight.astype(np.uint8)  # values 0..255 stored in int32
    p = np.arange(128)
    bm = (np.uint8(1) << (p % 8).astype(np.uint8))[:, None] * np.ones(
        (1, W_BIT * O_B), np.uint8)
    hm = (2.0 ** ((p % 8) - 1.0)).astype(np.float32).reshape(128, 1)
    pat = np.ascontiguousarray(np.broadcast_to(
        (2.0 ** (1.0 - (np.arange(512) % 8))).astype(np.float32), (16, 512)))
    idn = np.eye(128, dtype=np.float32)
    vt_c = np.ascontiguousarray(vt)
    in_maps = []
    for c in range(NCORES):
        sl = slice(c * O_SHARD, (c + 1) * O_SHARD)
        qbT = np.ascontiguousarray(
            qb.reshape(W_BIT, OUT_F, IN_F // 8)[:, sl, :].transpose(0, 2, 1))
        uT = np.ascontiguousarray(u[:, sl, :].transpose(0, 2, 1))
        in_maps.append({
            "xT": xT, "qbT": qbT, "uT": uT, "vt": vt_c,
            "bm": bm, "hm": hm, "pat": pat, "idn": idn,
        })
    return in_maps


def _enable_ldw_opt():
    """No-op (kept for test.py compatibility). The walrus ldw-opt pass is
    incompatible with this kernel's LDWEIGHTS stream; bf16 FWL + the PE's
    background weight buffer hide the reloads instead."""
    return


def kernel(x, qweight, u, vt):
    from concourse import bass_utils
    _enable_ldw_opt()
    x = np.asarray(x)
    qweight = np.asarray(qweight)
    u = np.asarray(u)
    vt = np.asarray(vt)
    B, S, _ = x.shape
    M = B * S
    nc = build_bass(M)
    in_maps = prep_inputs(x, qweight, u, vt)
    res = bass_utils.run_bass_kernel_spmd(nc, in_maps, core_ids=list(range(NCORES)))
    out = np.empty((M, OUT_F), np.float32)
    for c in range(NCORES):
        out[:, c * O_SHARD:(c + 1) * O_SHARD] = res.results[c]["outT"].T
    return out.reshape(B, S, OUT_F)


if __name__ == "__main__":
    rng = np.random.default_rng(0)
    x = rng.standard_normal((4, 2048, IN_F)).astype(np.float32)
    qw = rng.integers(0, 256, size=(W_BIT, OUT_F * IN_F // 8)).astype(np.int32)
    uu = (rng.standard_normal((W_BIT, OUT_F, RANK)) * 0.05).astype(np.float32)
    vv = (rng.standard_normal((W_BIT, RANK, IN_F)) * 0.05).astype(np.float32)
    out = kernel(x=x, qweight=qw, u=uu, vt=vv)
    print(out.shape, out.dtype)


# revision 13
# speedup vs baseline: 1.5736x; 1.1893x over previous
"""BitStackLinear Trainium2 kernel (v2: bf16 GEMM with SBUF-resident w.T).

Computes out = x @ w.T where w = sum_i sign_i * (u_i @ vt_i), signs unpacked
from 4 packed bit-planes (one byte = 8 signs, little-endian).

Strategy: tensor-parallel over out_features across 8 NeuronCores
(1376 rows each). Per core, the o-dim is split into chunk A (4 o-tiles,
512 cols) and chunk B (7 o-tiles, 864 cols) so that reconstruction of B
overlaps the GEMM over A:

  [recon A] -> [GEMM-A over all m  ||  recon B] -> [GEMM-B over all m]

Reconstruction of w.T chunk (per 128-row k-slab, bf16, RESIDENT in SBUF):
  - DMA: vt k-slices (4 bits packed in one [16,512] tile); packed sign
    bytes broadcast 8x across partitions (4 bits side by side)
  - GpSimd: vtb4s = vtb4 * pat (folds the 2^(1-j) per-k scale, j=k%8);
    a4 = bytes4 & (1<<j) in {0, 2^j} (one i32 AND for all 4 bits)
  - PE: pr_i = vtb4s_i.T @ u_i.T -> PSUM f32 (rank-16 matmuls)
  - DVE: t_i = (a_i - 2^(j-1)) * pr_i = sign_i * r_i (STT, bf16 out)
  - PE: acc += I.T @ t_i (identity matmuls accumulate the 4 bit-planes in
    f32 PSUM; replaces 3 DVE adds)
  - ScalarE: wt[ks] = acc (evacuate to the resident bf16 w.T tile)
GEMM (all-bf16 PE, PSUM accumulation over all 32 k-slabs):
  - x.T streamed f32 per 512-col m-block, cast to bf16 (ScalarE/DVE/GpSimd)
  - stationary = resident wt[k] column tiles (bf16 -> FWL hides LDWEIGHTS)
  - ScalarE evacuation, DMA out

kernel(**inputs) takes the full unsharded inputs and returns the full output.
Host work is layout only: transposes, dtype reinterpretation, sharding.
"""

import numpy as np

import concourse.bass as bass
import concourse.bacc as bacc
import concourse.mybir as mybir
import concourse.tile as tile

W_BIT = 4
OUT_F = 11008
IN_F = 4096
RANK = 16
NCORES = 8
O_SHARD = OUT_F // NCORES          # 1376
O_TILES = (O_SHARD + 127) // 128   # 11 (last tile 96 wide)
K_TILES = IN_F // 128              # 32
MB = 512                           # m-block (x chunk width)
OT_A = 4                           # o-tiles in chunk A
O_A = OT_A * 128                   # 512
O_B = O_SHARD - O_A                # 864


def _bitstack_body(tc, aps, M):
    nc = tc.nc
    xT, qbT, uT, vt, bm, hm, pat, idn, outT = (
        aps["xT"], aps["qbT"], aps["uT"], aps["vt"], aps["bm"], aps["hm"],
        aps["pat"], aps["idn"], aps["outT"],
    )
    f32, u8, i32 = mybir.dt.float32, mybir.dt.uint8, mybir.dt.int32
    bf16 = mybir.dt.bfloat16
    n_mb = M // MB

    import contextlib
    with contextlib.ExitStack() as ctx:
        pool = ctx.enter_context(tc.tile_pool(name="sb", bufs=1))
        psum = ctx.enter_context(tc.tile_pool(name="ps", bufs=1, space="PSUM"))

        # ---- constants resident in SBUF ----
        bm_t = pool.tile([128, W_BIT * O_B], u8, name="bm_t")
        nc.sync.dma_start(bm_t, bm)
        hm_t = pool.tile([128, 1], f32, name="hm_t")
        nc.sync.dma_start(hm_t, hm)
        pat_t = pool.tile([16, 512], f32, name="pat_t")
        nc.sync.dma_start(pat_t, pat)
        idf_t = pool.tile([128, 128], f32, name="idf_t")
        nc.sync.dma_start(idf_t, idn)
        idn_t = pool.tile([128, 128], bf16, name="idn_t")
        nc.scalar.copy(idn_t, idf_t)

        # u.T resident in bf16: staged per 512-chunk through a small f32 tile
        utb = []
        for i in range(W_BIT):
            t = pool.tile([16, O_SHARD], bf16, name=f"utb{i}", tag="utb", bufs=4)
            for c0 in range(0, O_SHARD, 512):
                c1 = min(c0 + 512, O_SHARD)
                st = pool.tile([16, 512], f32, name=f"ust{i}_{c0}", tag="ost",
                               bufs=2)
                nc.sync.dma_start(st[:, :c1 - c0], uT[i, :, c0:c1])
                nc.scalar.copy(t[:, c0:c1], st[:, :c1 - c0])
            utb.append(t)

        # ---- x streaming helpers (DMA f32 chunk, cast to bf16) ----
        xbl = {}

        def emit_x(pas, mb, k, eng):
            xs = pool.tile([128, MB], f32, name=f"xs{pas}{mb}_{k}", tag="xs",
                           bufs=4)
            nc.sync.dma_start(xs, xT[k * 128:(k + 1) * 128,
                                     mb * MB:(mb + 1) * MB])
            xb = pool.tile([128, MB], bf16, name=f"xb{pas}{mb}_{k}", tag="xb",
                           bufs=60)
            if eng == "s":
                nc.scalar.copy(xb, xs)
            elif eng == "v":
                nc.vector.tensor_copy(out=xb, in_=xs)
            else:
                nc.gpsimd.tensor_copy(out=xb, in_=xs)
            xbl[(pas, mb, k)] = xb

        def recon_slab(ks, o0, ow, wtag, dmae, dmab):
            """Reconstruct w.T[ks*128:(ks+1)*128, o0:o0+ow] -> bf16 tile."""
            vtb4 = pool.tile([16, 512], f32, name=f"vtb4_{wtag}{ks}",
                             tag="vtb4", bufs=2)
            for i in range(W_BIT):
                dmae.dma_start(vtb4[:, i * 128:(i + 1) * 128],
                               vt[i, :, ks * 128:(ks + 1) * 128])
            vtb4s = pool.tile([16, 512], bf16, name=f"vtb4s_{wtag}{ks}",
                              tag="vtb4s", bufs=2)
            nc.gpsimd.tensor_tensor(out=vtb4s, in0=vtb4, in1=pat_t,
                                    op=mybir.AluOpType.mult)
            bts4 = pool.tile([128, W_BIT * ow], u8, name=f"bts{wtag}{ks}",
                             tag=f"bts{wtag}", bufs=1)
            for i in range(W_BIT):
                src = qbT[i, ks * 16:(ks + 1) * 16,
                          o0:o0 + ow][:, None, :].to_broadcast((16, 8, ow))
                dmab.dma_start(bts4[:, i * ow:(i + 1) * ow], src)
            a4 = pool.tile([128, W_BIT * ow], u8, name=f"a{wtag}{ks}",
                           tag=f"a{wtag}", bufs=2)
            nc.vector.tensor_tensor(out=a4.bitcast(i32), in0=bts4.bitcast(i32),
                                    in1=bm_t.bitcast(i32)[:, 0:W_BIT * ow // 4],
                                    op=mybir.AluOpType.bitwise_and)
            acc = psum.tile([128, O_B], f32, name=f"acc{wtag}{ks}", tag="acc",
                            bufs=1)
            chunks = [(c0, min(c0 + 512, ow)) for c0 in range(0, ow, 512)]
            for i in range(W_BIT):
                pr = psum.tile([128, O_B], f32, name=f"pr{wtag}{ks}_{i}",
                               tag="pr", bufs=2)
                for c0, c1 in chunks:
                    nc.tensor.matmul(pr[:, c0:c1],
                                     vtb4s[:, i * 128:(i + 1) * 128],
                                     utb[i][:, o0 + c0:o0 + c1],
                                     start=True, stop=True)
                t_t = pool.tile([128, O_B], bf16, name=f"t{wtag}{ks}_{i}",
                                tag="tt", bufs=2)
                nc.vector.scalar_tensor_tensor(
                    out=t_t[:, :ow], in0=a4[:, i * ow:(i + 1) * ow],
                    scalar=hm_t, in1=pr[:, :ow],
                    op0=mybir.AluOpType.subtract, op1=mybir.AluOpType.mult)
                for c0, c1 in chunks:
                    nc.tensor.matmul(acc[:, c0:c1], idn_t, t_t[:, c0:c1],
                                     start=(i == 0), stop=(i == W_BIT - 1))
            w = pool.tile([128, ow], bf16, name=f"wt{wtag}{ks}", tag=f"wt{wtag}",
                          bufs=K_TILES)
            nc.scalar.copy(w, acc[:, :ow])
            return w

        def gemm_mb(pas, mb, ots, wtl, o_base):
            for ot in ots:
                ow = min(128, O_SHARD - ot * 128)
                c0 = ot * 128 - o_base
                pg = psum.tile([128, MB], f32, name=f"g{pas}{mb}_{ot}",
                               tag="pg", bufs=2)
                for k in range(K_TILES):
                    nc.tensor.matmul(
                        pg[:ow],
                        wtl[k][:, c0:c0 + ow],
                        xbl[(pas, mb, k)],
                        start=(k == 0), stop=(k == K_TILES - 1),
                    )
                ost = pool.tile([128, MB], f32, name=f"ost{pas}{mb}_{ot}",
                                tag="ost", bufs=2)
                nc.scalar.copy(ost[:ow], pg[:ow])
                nc.sync.dma_start(
                    outT[ot * 128:ot * 128 + ow, mb * MB:(mb + 1) * MB],
                    ost[:ow])

        # ---- Phase R-A: reconstruct w.T chunk A; sprinkle mb0/mb1 x prep ----
        cast_jobs = [("A", mb, k) for mb in (0, 1) for k in range(K_TILES)]
        wtA = []
        for ks in range(K_TILES):
            wtA.append(recon_slab(ks, 0, O_A, "A", nc.sync, nc.sync))
            if cast_jobs:
                emit_x(*cast_jobs.pop(0), "s")
            if cast_jobs:
                emit_x(*cast_jobs.pop(0), "s")

        # ---- Phase G-A x R-B: GEMM chunk A overlapping recon of chunk B ----
        wtB = []
        for mb in range(n_mb):
            if mb + 2 < n_mb:
                for k in range(K_TILES):
                    emit_x("A", mb + 2, k, "s" if k % 8 < 5 else "v")
            gemm_mb("A", mb, range(OT_A), wtA, 0)
            for s in (2 * mb, 2 * mb + 1):
                if s < K_TILES:
                    wtB.append(recon_slab(s, O_A, O_B, "B", nc.gpsimd,
                                          nc.scalar))
        while len(wtB) < K_TILES:
            wtB.append(recon_slab(len(wtB), O_A, O_B, "B", nc.gpsimd,
                                  nc.scalar))

        # ---- Phase G-B: GEMM chunk B ----
        for k in range(K_TILES):
            emit_x("B", 0, k, "s" if k % 2 == 0 else "v")
        for k in range(K_TILES):
            emit_x("B", 1, k, "s" if k % 2 == 0 else "v")
        for mb in range(n_mb):
            if mb + 2 < n_mb:
                for k in range(K_TILES):
                    emit_x("B", mb + 2, k, "s" if k % 2 == 0 else "v")
            gemm_mb("B", mb, range(OT_A, O_TILES), wtB, O_A)


def build_bass(M=8192):
    nc = bacc.Bacc("TRN2", target_bir_lowering=False, debug=False)
    f32, u8 = mybir.dt.float32, mybir.dt.uint8
    aps = {}
    aps["xT"] = nc.dram_tensor("xT", [IN_F, M], f32, kind="ExternalInput").ap()
    aps["qbT"] = nc.dram_tensor("qbT", [W_BIT, IN_F // 8, O_SHARD], u8,
                                kind="ExternalInput").ap()
    aps["uT"] = nc.dram_tensor("uT", [W_BIT, RANK, O_SHARD], f32,
                               kind="ExternalInput").ap()
    aps["vt"] = nc.dram_tensor("vt", [W_BIT, RANK, IN_F], f32,
                               kind="ExternalInput").ap()
    aps["bm"] = nc.dram_tensor("bm", [128, W_BIT * O_B], u8,
                               kind="ExternalInput").ap()
    aps["hm"] = nc.dram_tensor("hm", [128, 1], f32, kind="ExternalInput").ap()
    aps["pat"] = nc.dram_tensor("pat", [16, 512], f32, kind="ExternalInput").ap()
    aps["idn"] = nc.dram_tensor("idn", [128, 128], f32, kind="ExternalInput").ap()
    aps["outT"] = nc.dram_tensor("outT", [O_SHARD, M], f32,
                                 kind="ExternalOutput").ap()
    with tile.TileContext(nc) as tc:
        _bitstack_body(tc, aps, M)
    nc.compile()
    return nc


def prep_inputs(x, qweight, u, vt):
    """Host-side layout prep (transposes / dtype views / sharding only)."""
    M = x.shape[0] * x.shape[1]
    xT = np.ascontiguousarray(x.reshape(M, IN_F).T)
    qb = qweight.astype(np.uint8)  # values 0..255 stored in int32
    p = np.arange(128)
    bm = (np.uint8(1) << (p % 8).astype(np.uint8))[:, None] * np.ones(
        (1, W_BIT * O_B), np.uint8)
    hm = (2.0 ** ((p % 8) - 1.0)).astype(np.float32).reshape(128, 1)
    pat = np.ascontiguousarray(np.broadcast_to(
        (2.0 ** (1.0 - (np.arange(512) % 8))).astype(np.float32), (16, 512)))
    idn = np.eye(128, dtype=np.float32)
    vt_c = np.ascontiguousarray(vt)
    in_maps = []
    for c in range(NCORES):
        sl = slice(c * O_SHARD, (c + 1) * O_SHARD)
        qbT = np.ascontiguousarray(
            qb.reshape(W_BIT, OUT_F, IN_F // 8)[:, sl, :].transpose(0, 2, 1))
        uT = np.ascontiguousarray(u[:, sl, :].transpose(0, 2, 1))
        in_maps.append({
            "xT": xT, "qbT": qbT, "uT": uT, "vt": vt_c,
            "bm": bm, "hm": hm, "pat": pat, "idn": idn,
        })
    return in_maps


def _enable_ldw_opt():
    """No-op (kept for test.py compatibility). The walrus ldw-opt pass is
    incompatible with this kernel's LDWEIGHTS stream; bf16 FWL + the PE's
    background weight buffer hide the reloads instead."""
    return


def kernel(x, qweight, u, vt):
    from concourse import bass_utils
    _enable_ldw_opt()
    x = np.asarray(x)
    qweight = np.asarray(qweight)
    u = np.asarray(u)
    vt = np.asarray(vt)
    B, S, _ = x.shape
    M = B * S
    nc = build_bass(M)
    in_maps = prep_inputs(x, qweight, u, vt)
    res = bass_utils.run_bass_kernel_spmd(nc, in_maps, core_ids=list(range(NCORES)))
    out = np.empty((M, OUT_F), np.float32)
    for c in range(NCORES):
        out[:, c * O_SHARD:(c + 1) * O_SHARD] = res.results[c]["outT"].T
    return out.reshape(B, S, OUT_F)


if __name__ == "__main__":
    rng = np.random.default_rng(0)
    x = rng.standard_normal((4, 2048, IN_F)).astype(np.float32)
    qw = rng.integers(0, 256, size=(W_BIT, OUT_F * IN_F // 8)).astype(np.int32)
    uu = (rng.standard_normal((W_BIT, OUT_F, RANK)) * 0.05).astype(np.float32)
    vv = (rng.standard_normal((W_BIT, RANK, IN_F)) * 0.05).astype(np.float32)
    out = kernel(x=x, qweight=qw, u=uu, vt=vv)
    print(out.shape, out.dtype)
